# revision 49
# baseline (speedup 1.0000x reference)
"""Distributed gaussian-mask attention for trn2 (8 NeuronCores, SPMD).

Problem: B=2, S=2048, H=1024, 16 heads, hd=64.
  q/k/v = x@W*, dif = q - k, score = exp(-0.5 * dif @ dif^T),
  prob = score * triu(ones,k=1), ctx = prob @ v, out = ctx @ Wo + bo.
  (bq/bk/bv are zeros by construction -- folded out; dif = x @ (Wq-Wk).)

Sharding (collective-free SPMD; cores never talk to each other):
  - Head parallel: core c owns heads (2c, 2c+1) = 128 feature columns of
    Wq/Wk/Wv.  Each core computes D^T = x@(Wq-Wk)c and V for ALL tokens
    of its 2 heads and runs the full (anti-)causal attention triangle
    locally, producing ctx^T [128 feat, 4096 tok].
  - Out-projection WITHOUT any collective: core c holds Wo rows
    [128c, 128c+128) and computes the PARTIAL product
    outT_c = Woc^T @ ctxT_c  [1024 out-feat, 4096 tok] in fp32.
    The HOST sums the 8 partials (and adds bo).  Removes the AllToAll
    that stalled the PE for ~47us per batch in the previous design.

Perf structure:
  - x is DMA'd in 512-token SLABS (all 8 feature chunks of one token
    range per DMA) so the first D/V projection -- and therefore the
    first attention iteration -- starts ~2us in, not after all of x.
  - Projections and out-projection matmuls are dripped into the
    attention instruction stream (the PE has slack vs ACT/DVE exp),
    keeping the PE dense so the HAM clock-gate stays released.
  - Scores for the two heads run CONCURRENTLY on the PE via row groups
    (K=64 at rows 0/64); ctx via col groups (M=64 at cols 0/64).
  - exp runs entirely on ACT (ACUT=1024; the DVE Schraudolph tail is
    compiled out), which balances ACT ~91us vs DVE ~90us and improves
    accuracy.  Diagonal blocks exp through a [128, 2, n] access pattern
    that reads exactly the two written score ranges -- no junk columns
    (junk PSUM reads are also fatal under CoreSim's race checks).
  - Every cross-engine producer->consumer edge created through the drip
    queue carries an explicit tile.add_dep_helper sync edge, and each
    query row's ctx accumulation opens with the full-width j=3 diagonal
    block so start=True clears the whole PSUM range: both were sources
    of first-execution races (reads of uninitialized/stale state that
    looked correct on re-runs).  CoreSim (bass_interp) verifies the
    final program reads no uninitialized memory.
  - Out-projection partials bounce PSUM->SBUF as bf16 (DVE; tail chunk
    alternates ACT/DVE) and DMA out on the sync ring; the scalar ring
    stays clear for ACT during attention.

Precision: x/Wd fp16, dT fp16, score PSUM fp32, prob bf16, V bf16,
  ctx PSUM fp32, ctx bf16, Wo bf16, out-partial bf16 (summed fp64 on
  host; bo added on host -- it is zero by construction anyway).
"""
import numpy as np
import ml_dtypes

import concourse.bass as bass
import concourse.bacc as bacc
import concourse.mybir as mybir
import concourse.tile as tile
from concourse.bass_utils import run_bass_kernel_spmd

FP = mybir.dt.float32
F16 = mybir.dt.float16
BF = mybir.dt.bfloat16
I32 = mybir.dt.int32
AF = mybir.ActivationFunctionType
ALU = mybir.AluOpType

NC = 8
B, S, H, NH, HD = 2, 2048, 1024, 16, 64
T = B * S            # 4096 tokens
QB = 512             # query block
KB = 128             # key block
NQB = S // QB        # 4 query blocks per batch
NKB = S // KB        # 16 key blocks per batch
NSLAB = T // QB      # 8 token slabs (512 tokens each)

# Two-phase Schraudolph fast-exp: exp(-0.5*x) ~ g1 + 0.704*g2 with
#   g_i = bitcast_f32(int32(A*x + B_i)); the 0.5 averaging weight is
#   folded into B (exponent -1), the second phase is offset half a
#   mantissa period.  Max rel err 0.76% (vs 3.0% single-phase).
EXP_A = float(np.float32(-0.5 * (1 << 23) / np.log(2)))
EXP_B1 = float(np.float32(127 * (1 << 23) - (1 << 23) - 426000.0))
EXP_B2 = float(np.float32(127 * (1 << 23) - (1 << 23) + (1 << 22) - 426000.0))
EXP_W2 = 0.704
ACUT = 1024          # ACT exps everything (DVE fast-exp tail disabled)

_cached = {}


def _build(dbg=False):
    nc = bacc.Bacc("TRN2", target_bir_lowering=False, debug=False, num_devices=NC)

    # xs: host-pre-shuffled x, col = slab*4096 + k_chunk*512 + token
    xs = nc.dram_tensor("xs", [128, NSLAB * 4096], F16, kind="ExternalInput")
    # Wdc/Wvc host-pre-shuffled to [128, k_chunk*128 + col]
    Wdc = nc.dram_tensor("Wdc", [128, H], F16, kind="ExternalInput")
    Wvc = nc.dram_tensor("Wvc", [128, H], F16, kind="ExternalInput")
    Woc = nc.dram_tensor("Woc", [128, H], BF, kind="ExternalInput")
    mask_d = nc.dram_tensor("maskbf", [128, 128], BF, kind="ExternalInput")
    outT = nc.dram_tensor("outT", [H, T], BF, kind="ExternalOutput")
    if dbg:
        dbg_dT = nc.dram_tensor("dbg_dT", [128, 512], F16,
                                kind="ExternalOutput")
        dbg_Vg = nc.dram_tensor("dbg_Vg", [128, 512], BF,
                                kind="ExternalOutput")
        dbg_ctxT = nc.dram_tensor("dbg_ctxT", [128, 2048], BF,
                                  kind="ExternalOutput")
        dbg_xa = nc.dram_tensor("dbg_xa", [128, 4096], F16,
                                kind="ExternalOutput")

    with tile.TileContext(nc) as tc:
        with (
            tc.tile_pool(name="res", bufs=1) as res,      # resident SBUF
            tc.tile_pool(name="stream", bufs=3) as strm,  # streamed SBUF
            tc.tile_pool(name="pss", bufs=2, space="PSUM") as pss,   # 2x2 banks
            tc.tile_pool(name="pcx", bufs=2, space="PSUM") as pcx,   # 2x1 banks
            tc.tile_pool(name="paux", bufs=2, space="PSUM") as paux,  # 2x1 banks
        ):
            # ---------------- x slab 0 + weights first ----------------
            # slab 0 leads both rings so the first projection (and the
            # first attention iteration) starts as early as possible.
            xa = res.tile([128, NSLAB * 4096], F16, tag="xa", name="xa")
            nc.sync.dma_start(xa[:, 0:2048], xs[:, 0:2048])
            nc.scalar.dma_start(xa[:, 2048:4096], xs[:, 2048:4096])
            wd_t = res.tile([128, 1024], F16, tag="wd")
            nc.sync.dma_start(wd_t[:], Wdc[:])
            wv_t = res.tile([128, 1024], F16, tag="wv")
            nc.scalar.dma_start(wv_t[:], Wvc[:])
            mask_t = res.tile([128, 128], BF, tag="mask")
            nc.sync.dma_start(mask_t[:], mask_d[:])
            wo_t = res.tile([128, 1024], BF, tag="wo")
            nc.scalar.dma_start(wo_t[:], Woc[:])

            # ---------------- rest of x, 512-token slabs ----------------
            # xa col layout: sg*4096 + k*512 + t   (k = feature chunk)
            for sg in range(1, NSLAB):
                o = sg * 4096
                nc.sync.dma_start(xa[:, o:o + 2048], xs[:, o:o + 2048])
                nc.scalar.dma_start(
                    xa[:, o + 2048:o + 4096], xs[:, o + 2048:o + 4096]
                )

            # resident projection outputs
            dT = [res.tile([128, 512], F16, tag=f"dT{i}", name=f"dT{i}")
                  for i in range(NSLAB)]              # D^T [feat, tok]
            Vg = [res.tile([128, 512], BF, tag=f"Vg{i}", name=f"Vg{i}")
                  for i in range(NSLAB)]              # V [tok, feat] 4 subtiles
            ctxT = [res.tile([128, 2048], BF, tag=f"ctxT{b}", name=f"ctxT{b}")
                    for b in range(B)]

            # ---------------- PE drip work queue ----------------
            work = []            # list of (cost_ns, closure) PE micro-ops

            def drip(k):
                """Drain up to k deferred PE micro-ops.  (A time-weighted
                700ns/iter budget was tried and measured SLOWER --
                158.9us vs 149.7us -- the eager count-based drain keeps
                the DMA-gated prologue fed better.)"""
                for _ in range(k):
                    if not work:
                        return
                    work.pop(0)[1]()

            pstate = {}
            dt_ci = [None] * NSLAB    # dT[sg] producer (DVE copy) insts
            vg_ci = [None] * NSLAB    # Vg[sg] producer insts

            def sdep(mm, *cis):
                """Explicit sync edges: Tile's scheduler misses some
                cross-engine RAW deps created through the drip queue
                (CoreSim: ctx matmul read Vg before its copy)."""
                for ci in cis:
                    assert ci is not None, "producer not yet emitted"
                    tile.add_dep_helper(mm.ins, ci.ins, sync=True,
                                        reason="producer copy done")

            def ensure(*slabs):
                """Drain drip items until every slab's dT/Vg producer
                copy has been emitted (so consumers can depend on it)."""
                while any(dt_ci[s] is None or vg_ci[s] is None
                          for s in slabs):
                    assert work, "work queue exhausted before producers"
                    work.pop(0)[1]()

            def proj_items(sg):
                """D then V projection micro-ops for one 512-token slab."""
                items = []

                def d_mm(k, sg=sg):
                    if k == 0:
                        pstate[f"pd{sg}"] = paux.tile(
                            [128, 512], FP, tag="pa", name=f"pd{sg}")
                    pd = pstate[f"pd{sg}"]
                    nc.tensor.matmul(
                        pd[:], wd_t[:, k * 128:(k + 1) * 128],
                        xa[:, sg * 4096 + k * 512:sg * 4096 + (k + 1) * 512],
                        start=(k == 0), stop=(k == 7), skip_group_check=True,
                    )
                    if k == 7:
                        dt_ci[sg] = nc.vector.tensor_copy(dT[sg][:], pd[:])

                def v_mm(t, k, sg=sg):
                    if t == 0 and k == 0:
                        pstate[f"pv{sg}"] = paux.tile(
                            [128, 512], FP, tag="pa", name=f"pv{sg}")
                    pv = pstate[f"pv{sg}"]
                    nc.tensor.matmul(
                        pv[:, t * 128:(t + 1) * 128],
                        xa[:, sg * 4096 + k * 512 + t * 128:
                           sg * 4096 + k * 512 + (t + 1) * 128],
                        wv_t[:, k * 128:(k + 1) * 128],
                        start=(k == 0), stop=(k == 7), skip_group_check=True,
                    )
                    if t == 3 and k == 7:
                        vg_ci[sg] = nc.vector.tensor_copy(Vg[sg][:], pv[:])

                for k in range(8):
                    items.append((250, lambda k=k: d_mm(k)))
                for t in range(4):
                    for k in range(8):
                        items.append((130, lambda t=t, k=k: v_mm(t, k)))
                return items

            def outproj_items(b, qb, dep=None, tail=False):
                """Partial out-projection for one 512-token ctx chunk.

                During attention all output DMAs ride the sync ring (the
                scalar queue must stay clear for ACT exp); the tail chunk
                alternates rings since ACT is done by then.
                """
                items = []

                def o_mm(fo, b=b, qb=qb):
                    po = paux.tile([128, 512], FP, tag="pa",
                                   name=f"po{b}_{qb}_{fo}")
                    mi = nc.tensor.matmul(
                        po[:], wo_t[:, fo * 128:(fo + 1) * 128],
                        ctxT[b][:, qb * 512:(qb + 1) * 512],
                        start=True, stop=True, skip_group_check=True,
                    )
                    if dep is not None:
                        # explicit sync edge: the ctxT chunk cast (DVE)
                        # must complete before this PE read -- the
                        # implicit transitive coverage proved racy.
                        tile.add_dep_helper(mi.ins, dep.ins, sync=True,
                                            reason="ctxT chunk ready")
                    # PSUM can't feed DMA: bounce through SBUF as bf16,
                    # alternating the cast between ACT and DVE
                    ot = strm.tile([128, 512], BF, tag="ot", bufs=3,
                                   name=f"ot{b}_{qb}_{fo}")
                    if fo % 2:   # ACT idles in outproj-heavy windows
                        nc.scalar.copy(ot[:], po[:])
                    else:
                        nc.vector.tensor_copy(ot[:], po[:])
                    # alternate rings: the po->DMA->paux-WAR chain on one
                    # ring head-of-line-blocked the PE queue for ~9us in
                    # late b1 (ACT sat idle waiting for scores); the
                    # ~620ns/DMA scalar-queue cost is absorbed by that
                    # same ACT idle.
                    eng = nc.scalar if fo % 2 else nc.sync
                    eng.dma_start(
                        outT[fo * 128:(fo + 1) * 128,
                             b * S + qb * 512:b * S + (qb + 1) * 512],
                        ot[:],
                    )

                for fo in range(8):
                    items.append((250, lambda fo=fo: o_mm(fo)))
                return items

            # ---- score pair (both heads, concurrent row tiles) ----
            # One fused PSUM tile [128, 1024]: head0 scores in cols
            # [0:512), head1 in [512:1024) (PSUM-bank aligned).
            def emit_score(b, qb, kb):
                qt = b * 4 + qb
                koff = b * S + kb * KB
                kt, kc = koff // 512, koff % 512
                j = kb - 4 * qb
                n = 128 * (j + 1) if j < 4 else QB
                ps = pss.tile([128, 2 * QB], FP, tag="ps",
                              name=f"ps_{b}_{qb}_{kb}")
                m1 = nc.tensor.matmul(
                    ps[:, 0:n], dT[kt][0:64, kc:kc + 128],
                    dT[qt][0:64, 0:n], start=True, stop=True,
                )
                sdep(m1, dt_ci[kt], dt_ci[qt])
                m2 = nc.tensor.matmul(
                    ps[:, QB:QB + n], dT[kt][64:128, kc:kc + 128],
                    dT[qt][64:128, 0:n], start=True, stop=True,
                    skip_group_check=True,
                )
                sdep(m2, dt_ci[kt], dt_ci[qt])
                return ps, j, n

            # ---- exp: one ACT instr; DVE two-phase fast-exp tail ----
            def emit_exp(b, qb, kb, ps, j, n):
                at = strm.tile([128, 2 * QB], BF, tag="at", bufs=3,
                               name=f"at_{b}_{qb}_{kb}")
                if n < QB:
                    # diagonal block, 2n <= 768 <= ACUT: one ACT over a
                    # [128, 2, n] view -- reads exactly the two written
                    # ranges [0:n) and [512:512+n), no junk columns.
                    nc.scalar.activation(
                        at[:].rearrange("p (a b) -> p a b", a=2)[:, :, 0:n],
                        ps[:].rearrange("p (a b) -> p a b", a=2)[:, :, 0:n],
                        AF.Exp, scale=-0.5,
                    )
                else:
                    nc.scalar.activation(at[:, 0:ACUT], ps[:, 0:ACUT],
                                         AF.Exp, scale=-0.5)
                if QB + n > ACUT:                 # DVE tail, 2-phase
                    w = QB + n - ACUT
                    i1 = strm.tile([128, 192], I32, tag="i1", bufs=2,
                                   name=f"i1_{b}_{qb}_{kb}")
                    i2 = strm.tile([128, 192], I32, tag="i2", bufs=2,
                                   name=f"i2_{b}_{qb}_{kb}")
                    nc.vector.tensor_scalar(
                        i1[:, 0:w], ps[:, ACUT:QB + n], EXP_A, EXP_B1,
                        ALU.mult, ALU.add,
                    )
                    nc.vector.tensor_scalar(
                        i2[:, 0:w], ps[:, ACUT:QB + n], EXP_A, EXP_B2,
                        ALU.mult, ALU.add,
                    )
                    nc.vector.scalar_tensor_tensor(
                        at[:, ACUT:QB + n], i2[:, 0:w].bitcast(FP), EXP_W2,
                        i1[:, 0:w].bitcast(FP), ALU.mult, ALU.add,
                    )
                if j < 4:                         # diagonal: mask last 128
                    nc.vector.tensor_mul(
                        at[:, j * 128:n], at[:, j * 128:n], mask_t[:]
                    )
                    nc.vector.tensor_mul(
                        at[:, QB + j * 128:QB + n],
                        at[:, QB + j * 128:QB + n], mask_t[:]
                    )
                return at

            # ---- ctx pair (both heads, concurrent col tiles) ----
            def emit_ctx(b, qb, kb, pc, at, n, first, last):
                g, go = (b * 16 + kb) // 4, ((b * 16 + kb) % 4) * 128
                m1 = nc.tensor.matmul(
                    pc[0:64, 0:n], Vg[g][:, go:go + 64], at[:, 0:n],
                    start=first, stop=last,
                    tile_position=(0, 0), skip_group_check=True,
                )
                sdep(m1, vg_ci[g])
                m2 = nc.tensor.matmul(
                    pc[64:128, 0:n], Vg[g][:, go + 64:go + 128],
                    at[:, QB:QB + n],
                    start=first, stop=last,
                    tile_position=(0, 64), skip_group_check=True,
                )
                sdep(m2, vg_ci[g])

            # ---------------- prologue: slab 0 projections --------------
            for _, it in proj_items(0):
                it()
            # remaining slabs go through the drip queue (b0's own slabs
            # 1-3 first; they gate early attention iterations, so the
            # early budget is generous)
            for sg in range(1, NSLAB):
                work.extend(proj_items(sg))

            # ---------------- attention main loop, software-pipelined ----
            # Per query row, the diagonal j=3 block (n=512, full width)
            # runs FIRST so the start=True ctx matmul clears the whole
            # 512-col PSUM range: PSUM has_written bits are in an
            # arbitrary state on first execution, and narrower start
            # blocks left cols the later accumulating matmuls touched
            # uninitialized (NaN ctx on the first-ever query row).
            for b in range(B):
                pend = None                       # (qb, kb, first, last)+score
                pc = None
                for qb in range(NQB):
                    seq = ([4 * qb + 3, 4 * qb, 4 * qb + 1, 4 * qb + 2]
                           + list(range(4 * qb + 4, NKB)))
                    for idx, kb in enumerate(seq):
                        fl = (idx == 0, idx == len(seq) - 1)
                        ensure(b * 4 + qb, b * 4 + kb // 4)
                        if pend is None:          # prologue of this batch
                            pend = (qb, kb) + fl + emit_score(b, qb, kb)
                            pc = pcx.tile([128, QB], FP, tag="pc",
                                          name=f"pc{b}_{qb}")
                            drip(6)
                            continue
                        pqb, pkb, pfirst, plast, ps, j, n = pend
                        at = emit_exp(b, pqb, pkb, ps, j, n)
                        # next score pair ahead of this ctx pair
                        pend = (qb, kb) + fl + emit_score(b, qb, kb)
                        if qb != pqb:             # new q row -> new psum
                            pc_next = pcx.tile([128, QB], FP, tag="pc",
                                               name=f"pc{b}_{qb}")
                        emit_ctx(b, pqb, pkb, pc, at, n, pfirst, plast)
                        if qb != pqb:
                            ci = nc.vector.tensor_copy(
                                ctxT[b][:, pqb * QB:(pqb + 1) * QB], pc[:]
                            )
                            work.extend(outproj_items(b, pqb, dep=ci))
                            pc = pc_next
                        # generous budget while projections are pending
                        # (gated by x DMA anyway); 3/iter in steady state
                        drip(9 if b == 0 else 3)
                # drain the last pending iteration
                pqb, pkb, pfirst, plast, ps, j, n = pend
                at = emit_exp(b, pqb, pkb, ps, j, n)
                emit_ctx(b, pqb, pkb, pc, at, n, pfirst, plast)
                ci = nc.vector.tensor_copy(
                    ctxT[b][:, pqb * QB:(pqb + 1) * QB], pc[:]
                )
                work.extend(outproj_items(b, pqb, dep=ci, tail=(b == 1)))

            # tail: whatever is still queued (last out-proj chunk)
            while work:
                work.pop(0)[1]()

            # Completion barrier for the final output DMAs: nothing
            # re-reads outT on-device, so the last ot buffers' DMA
            # semaphores would otherwise never be waited on and the
            # program could "finish" with writes still in flight
            # (observed as garbage tail chunks on first execution).
            # Re-allocating every ot buffer forces a WAR wait on each
            # outstanding DMA's completion semaphore.
            for i in range(3):
                fin = strm.tile([128, 512], BF, tag="ot", bufs=3,
                                name=f"fin{i}")
                nc.vector.tensor_copy(fin[:, 0:8], mask_t[:, 0:8])

            if dbg:
                nc.sync.dma_start(dbg_dT[:], dT[0][:])
                nc.sync.dma_start(dbg_Vg[:], Vg[0][:])
                nc.sync.dma_start(dbg_ctxT[:], ctxT[0][:])
                nc.sync.dma_start(dbg_xa[:], xa[:, 0:4096])
                dchk = res.tile([128, 8], BF, tag="dchk")
                nc.scalar.dma_start(dchk[:], dbg_ctxT[:, 0:8])
                dchk2 = res.tile([128, 8], BF, tag="dchk2")
                nc.vector.tensor_copy(dchk2[:], dchk[:])

    nc.compile()
    return nc


def make_in_maps(inputs):
    x = np.asarray(inputs["x"], np.float32)
    Wq = np.asarray(inputs["Wq"], np.float32)
    Wk = np.asarray(inputs["Wk"], np.float32)
    Wv = np.asarray(inputs["Wv"], np.float32)
    Wo = np.asarray(inputs["Wo"], np.float32)
    # bq/bk/bv are zeros by the problem's input spec; dif = x @ (Wq - Wk)
    # and v = x @ Wv absorb them exactly when zero.  bo is added on host.

    # xs[p, sg*4096 + k*512 + t] = x[token sg*512+t, feature k*128+p]
    xT = x.reshape(T, H).T                       # [feat, tok]
    xs = np.ascontiguousarray(
        xT.reshape(8, 128, NSLAB, 512).transpose(1, 2, 0, 3).reshape(
            128, NSLAB * 4096)
    ).astype(np.float16)
    Wd = Wq - Wk
    maskbf = np.tril(np.ones((128, 128), np.float32), -1).astype(
        ml_dtypes.bfloat16)

    def chunkify(w):                             # [1024, 128] -> [128, 1024]
        return np.ascontiguousarray(
            w.reshape(8, 128, 128).transpose(1, 0, 2).reshape(128, 1024))

    in_maps = []
    for c in range(NC):
        cols = slice(c * 128, (c + 1) * 128)
        in_maps.append({
            "xs": xs,
            "Wdc": chunkify(Wd[:, cols]).astype(np.float16),
            "Wvc": chunkify(Wv[:, cols]).astype(np.float16),
            "Woc": np.ascontiguousarray(Wo[cols, :]).astype(
                ml_dtypes.bfloat16),
            "maskbf": maskbf,
        })
    return in_maps


def gather_out(res, bo):
    acc = np.zeros((H, T), np.float64)
    for c in range(NC):
        acc += np.asarray(res.results[c]["outT"], np.float32)
    return acc.T.reshape(B, S, H).astype(np.float32) + bo


def kernel(**inputs):
    if "nc" not in _cached:
        _cached["nc"] = _build()
    nc = _cached["nc"]
    in_maps = make_in_maps(inputs)
    res = run_bass_kernel_spmd(nc, in_maps, core_ids=list(range(NC)))
    return gather_out(res, np.asarray(inputs["bo"], np.float32))


# revision 51
# speedup vs baseline: 1.0941x; 1.0941x over previous
"""Distributed gaussian-mask attention for trn2 (8 NeuronCores, SPMD).

Problem: B=2, S=2048, H=1024, 16 heads, hd=64.
  q/k/v = x@W*, dif = q - k, score = exp(-0.5 * dif @ dif^T),
  prob = score * triu(ones,k=1), ctx = prob @ v, out = ctx @ Wo + bo.
  (bq/bk/bv are zeros by construction -- folded out; dif = x @ (Wq-Wk).)

Sharding (collective-free SPMD; cores never talk to each other):
  - Head parallel: core c owns heads (2c, 2c+1) = 128 feature columns of
    Wq/Wk/Wv.  Each core computes D^T = x@(Wq-Wk)c and V for ALL tokens
    of its 2 heads and runs the full (anti-)causal attention triangle
    locally, producing ctx^T [128 feat, 4096 tok].
  - Out-projection WITHOUT any collective: core c holds Wo rows
    [128c, 128c+128) and computes the PARTIAL product
    outT_c = Woc^T @ ctxT_c  [1024 out-feat, 4096 tok] in fp32.
    The HOST sums the 8 partials (and adds bo).  Removes the AllToAll
    that stalled the PE for ~47us per batch in the previous design.

Perf structure:
  - x is DMA'd in 512-token SLABS (all 8 feature chunks of one token
    range per DMA) so the first D/V projection -- and therefore the
    first attention iteration -- starts ~2us in, not after all of x.
  - Projections and out-projection matmuls are dripped into the
    attention instruction stream (the PE has slack vs ACT/DVE exp),
    keeping the PE dense so the HAM clock-gate stays released.
  - Scores for the two heads run CONCURRENTLY on the PE via row groups
    (K=64 at rows 0/64); ctx via col groups (M=64 at cols 0/64).
  - exp runs entirely on ACT (ACUT=1024; the DVE Schraudolph tail is
    compiled out), which balances ACT ~91us vs DVE ~90us and improves
    accuracy.  Diagonal blocks exp through a [128, 2, n] access pattern
    that reads exactly the two written score ranges -- no junk columns
    (junk PSUM reads are also fatal under CoreSim's race checks).
  - Every cross-engine producer->consumer edge created through the drip
    queue carries an explicit tile.add_dep_helper sync edge, and each
    query row's ctx accumulation opens with the full-width j=3 diagonal
    block so start=True clears the whole PSUM range: both were sources
    of first-execution races (reads of uninitialized/stale state that
    looked correct on re-runs).  CoreSim (bass_interp) verifies the
    final program reads no uninitialized memory.
  - Out-projection partials bounce PSUM->SBUF as bf16 (DVE; tail chunk
    alternates ACT/DVE) and DMA out on the sync ring; the scalar ring
    stays clear for ACT during attention.

Precision: x/Wd fp16, dT fp16, score PSUM fp32, prob bf16, V bf16,
  ctx PSUM fp32, ctx bf16, Wo bf16, out-partial bf16 (summed fp64 on
  host; bo added on host -- it is zero by construction anyway).
"""
import numpy as np
import ml_dtypes

import concourse.bass as bass
import concourse.bacc as bacc
import concourse.mybir as mybir
import concourse.tile as tile
from concourse.bass_utils import run_bass_kernel_spmd

FP = mybir.dt.float32
F16 = mybir.dt.float16
BF = mybir.dt.bfloat16
I32 = mybir.dt.int32
AF = mybir.ActivationFunctionType
ALU = mybir.AluOpType

NC = 8
B, S, H, NH, HD = 2, 2048, 1024, 16, 64
T = B * S            # 4096 tokens
QB = 512             # query block
KB = 128             # key block
NQB = S // QB        # 4 query blocks per batch
NKB = S // KB        # 16 key blocks per batch
NSLAB = T // QB      # 8 token slabs (512 tokens each)

# Two-phase Schraudolph fast-exp: exp(-0.5*x) ~ g1 + 0.704*g2 with
#   g_i = bitcast_f32(int32(A*x + B_i)); the 0.5 averaging weight is
#   folded into B (exponent -1), the second phase is offset half a
#   mantissa period.  Max rel err 0.76% (vs 3.0% single-phase).
EXP_A = float(np.float32(-0.5 * (1 << 23) / np.log(2)))
EXP_B1 = float(np.float32(127 * (1 << 23) - (1 << 23) - 426000.0))
EXP_B2 = float(np.float32(127 * (1 << 23) - (1 << 23) + (1 << 22) - 426000.0))
EXP_W2 = 0.704
ACUT = 1024          # ACT exps everything (DVE fast-exp tail disabled)

_cached = {}


def _build(dbg=False):
    nc = bacc.Bacc("TRN2", target_bir_lowering=False, debug=False, num_devices=NC)

    # xs: host-pre-shuffled x, col = slab*4096 + k_chunk*512 + token
    xs = nc.dram_tensor("xs", [128, NSLAB * 4096], F16, kind="ExternalInput")
    # Wdc/Wvc host-pre-shuffled to [128, k_chunk*128 + col]
    Wdc = nc.dram_tensor("Wdc", [128, H], F16, kind="ExternalInput")
    Wvc = nc.dram_tensor("Wvc", [128, H], F16, kind="ExternalInput")
    Woc = nc.dram_tensor("Woc", [128, H], BF, kind="ExternalInput")
    mask_d = nc.dram_tensor("maskbf", [128, 128], BF, kind="ExternalInput")
    outT = nc.dram_tensor("outT", [H, T], BF, kind="ExternalOutput")
    if dbg:
        dbg_dT = nc.dram_tensor("dbg_dT", [128, 512], F16,
                                kind="ExternalOutput")
        dbg_Vg = nc.dram_tensor("dbg_Vg", [128, 512], BF,
                                kind="ExternalOutput")
        dbg_ctxT = nc.dram_tensor("dbg_ctxT", [128, 2048], BF,
                                  kind="ExternalOutput")
        dbg_xa = nc.dram_tensor("dbg_xa", [128, 4096], F16,
                                kind="ExternalOutput")

    with tile.TileContext(nc) as tc:
        with (
            tc.tile_pool(name="res", bufs=1) as res,      # resident SBUF
            tc.tile_pool(name="stream", bufs=3) as strm,  # streamed SBUF
            tc.tile_pool(name="pss", bufs=2, space="PSUM") as pss,   # 2x2 banks
            tc.tile_pool(name="pcx", bufs=2, space="PSUM") as pcx,   # 2x1 banks
            tc.tile_pool(name="paux", bufs=2, space="PSUM") as paux,  # 2x1 banks
        ):
            # ---------------- x slab 0 + weights first ----------------
            # slab 0 leads both rings so the first projection (and the
            # first attention iteration) starts as early as possible.
            xa = res.tile([128, NSLAB * 4096], F16, tag="xa", name="xa")
            nc.sync.dma_start(xa[:, 0:2048], xs[:, 0:2048])
            nc.scalar.dma_start(xa[:, 2048:4096], xs[:, 2048:4096])
            wd_t = res.tile([128, 1024], F16, tag="wd")
            nc.sync.dma_start(wd_t[:], Wdc[:])
            wv_t = res.tile([128, 1024], F16, tag="wv")
            nc.scalar.dma_start(wv_t[:], Wvc[:])
            mask_t = res.tile([128, 128], BF, tag="mask")
            nc.sync.dma_start(mask_t[:], mask_d[:])
            wo_t = res.tile([128, 1024], BF, tag="wo")
            nc.scalar.dma_start(wo_t[:], Woc[:])

            # ---------------- rest of x, 512-token slabs ----------------
            # xa col layout: sg*4096 + k*512 + t   (k = feature chunk)
            for sg in range(1, NSLAB):
                o = sg * 4096
                nc.sync.dma_start(xa[:, o:o + 2048], xs[:, o:o + 2048])
                nc.scalar.dma_start(
                    xa[:, o + 2048:o + 4096], xs[:, o + 2048:o + 4096]
                )

            # resident projection outputs
            dT = [res.tile([128, 512], F16, tag=f"dT{i}", name=f"dT{i}")
                  for i in range(NSLAB)]              # D^T [feat, tok]
            Vg = [res.tile([128, 512], BF, tag=f"Vg{i}", name=f"Vg{i}")
                  for i in range(NSLAB)]              # V [tok, feat] 4 subtiles
            ctxT = [res.tile([128, 2048], BF, tag=f"ctxT{b}", name=f"ctxT{b}")
                    for b in range(B)]

            # ---------------- PE drip work queue ----------------
            work = []            # list of (cost_ns, closure) PE micro-ops

            def drip(k):
                """Drain up to k deferred PE micro-ops.  (A time-weighted
                700ns/iter budget was tried and measured SLOWER --
                158.9us vs 149.7us -- the eager count-based drain keeps
                the DMA-gated prologue fed better.)"""
                for _ in range(k):
                    if not work:
                        return
                    work.pop(0)[1]()

            pstate = {}
            dt_ci = [None] * NSLAB    # dT[sg] producer (DVE copy) insts
            vg_ci = [None] * NSLAB    # Vg[sg] producer insts

            def sdep(mm, *cis):
                """Explicit sync edges: Tile's scheduler misses some
                cross-engine RAW deps created through the drip queue
                (CoreSim: ctx matmul read Vg before its copy)."""
                for ci in cis:
                    assert ci is not None, "producer not yet emitted"
                    tile.add_dep_helper(mm.ins, ci.ins, sync=True,
                                        reason="producer copy done")

            def ensure(*slabs):
                """Drain drip items until every slab's dT/Vg producer
                copy has been emitted (so consumers can depend on it)."""
                while any(dt_ci[s] is None or vg_ci[s] is None
                          for s in slabs):
                    assert work, "work queue exhausted before producers"
                    work.pop(0)[1]()

            def proj_items(sg):
                """D then V projection micro-ops for one 512-token slab."""
                items = []

                def d_mm(k, sg=sg):
                    if k == 0:
                        pstate[f"pd{sg}"] = paux.tile(
                            [128, 512], FP, tag="pa", name=f"pd{sg}")
                    pd = pstate[f"pd{sg}"]
                    nc.tensor.matmul(
                        pd[:], wd_t[:, k * 128:(k + 1) * 128],
                        xa[:, sg * 4096 + k * 512:sg * 4096 + (k + 1) * 512],
                        start=(k == 0), stop=(k == 7), skip_group_check=True,
                    )
                    if k == 7:
                        dt_ci[sg] = nc.vector.tensor_copy(dT[sg][:], pd[:])

                def v_mm(t, k, sg=sg):
                    if t == 0 and k == 0:
                        pstate[f"pv{sg}"] = paux.tile(
                            [128, 512], FP, tag="pa", name=f"pv{sg}")
                    pv = pstate[f"pv{sg}"]
                    nc.tensor.matmul(
                        pv[:, t * 128:(t + 1) * 128],
                        xa[:, sg * 4096 + k * 512 + t * 128:
                           sg * 4096 + k * 512 + (t + 1) * 128],
                        wv_t[:, k * 128:(k + 1) * 128],
                        start=(k == 0), stop=(k == 7), skip_group_check=True,
                    )
                    if t == 3 and k == 7:
                        vg_ci[sg] = nc.vector.tensor_copy(Vg[sg][:], pv[:])

                for k in range(8):
                    items.append((250, lambda k=k: d_mm(k)))
                for t in range(4):
                    for k in range(8):
                        items.append((130, lambda t=t, k=k: v_mm(t, k)))
                return items

            def outproj_items(b, qb, dep=None, tail=False):
                """Partial out-projection for one 512-token ctx chunk.

                During attention all output DMAs ride the sync ring (the
                scalar queue must stay clear for ACT exp); the tail chunk
                alternates rings since ACT is done by then.
                """
                items = []

                def o_mm(fo, b=b, qb=qb):
                    po = paux.tile([128, 512], FP, tag="pa",
                                   name=f"po{b}_{qb}_{fo}")
                    mi = nc.tensor.matmul(
                        po[:], wo_t[:, fo * 128:(fo + 1) * 128],
                        ctxT[b][:, qb * 512:(qb + 1) * 512],
                        start=True, stop=True, skip_group_check=True,
                    )
                    if dep is not None:
                        # explicit sync edge: the ctxT chunk cast (DVE)
                        # must complete before this PE read -- the
                        # implicit transitive coverage proved racy.
                        tile.add_dep_helper(mi.ins, dep.ins, sync=True,
                                            reason="ctxT chunk ready")
                    # PSUM can't feed DMA: bounce through SBUF as bf16,
                    # alternating the cast between ACT and DVE
                    ot = strm.tile([128, 512], BF, tag="ot", bufs=3,
                                   name=f"ot{b}_{qb}_{fo}")
                    # All attention-phase casts stay on DVE: routing the
                    # odd-fo casts to ACT was measured SLOWER (164.2us
                    # vs 149.1us) -- the Scalar queue is strict FIFO, so
                    # copies delay the score->exp->ctx chain even though
                    # ACT has idle windows.  Only the tail (no more exps
                    # pending) alternates.
                    if tail and fo % 2:
                        nc.scalar.copy(ot[:], po[:])
                    else:
                        nc.vector.tensor_copy(ot[:], po[:])
                    # alternate rings: the po->DMA->paux-WAR chain on one
                    # ring head-of-line-blocked the PE queue for ~9us in
                    # late b1 (ACT sat idle waiting for scores); the
                    # ~620ns/DMA scalar-queue cost is absorbed by that
                    # same ACT idle.
                    eng = nc.scalar if fo % 2 else nc.sync
                    eng.dma_start(
                        outT[fo * 128:(fo + 1) * 128,
                             b * S + qb * 512:b * S + (qb + 1) * 512],
                        ot[:],
                    )

                for fo in range(8):
                    items.append((250, lambda fo=fo: o_mm(fo)))
                return items

            # ---- score pair (both heads, concurrent row tiles) ----
            # One fused PSUM tile [128, 1024]: head0 scores in cols
            # [0:512), head1 in [512:1024) (PSUM-bank aligned).
            def emit_score(b, qb, kb):
                qt = b * 4 + qb
                koff = b * S + kb * KB
                kt, kc = koff // 512, koff % 512
                j = kb - 4 * qb
                n = 128 * (j + 1) if j < 4 else QB
                ps = pss.tile([128, 2 * QB], FP, tag="ps",
                              name=f"ps_{b}_{qb}_{kb}")
                m1 = nc.tensor.matmul(
                    ps[:, 0:n], dT[kt][0:64, kc:kc + 128],
                    dT[qt][0:64, 0:n], start=True, stop=True,
                )
                sdep(m1, dt_ci[kt], dt_ci[qt])
                m2 = nc.tensor.matmul(
                    ps[:, QB:QB + n], dT[kt][64:128, kc:kc + 128],
                    dT[qt][64:128, 0:n], start=True, stop=True,
                    skip_group_check=True,
                )
                sdep(m2, dt_ci[kt], dt_ci[qt])
                return ps, j, n

            # ---- exp: one ACT instr; DVE two-phase fast-exp tail ----
            def emit_exp(b, qb, kb, ps, j, n):
                at = strm.tile([128, 2 * QB], BF, tag="at", bufs=3,
                               name=f"at_{b}_{qb}_{kb}")
                if n < QB:
                    # diagonal block, 2n <= 768 <= ACUT: one ACT over a
                    # [128, 2, n] view -- reads exactly the two written
                    # ranges [0:n) and [512:512+n), no junk columns.
                    nc.scalar.activation(
                        at[:].rearrange("p (a b) -> p a b", a=2)[:, :, 0:n],
                        ps[:].rearrange("p (a b) -> p a b", a=2)[:, :, 0:n],
                        AF.Exp, scale=-0.5,
                    )
                else:
                    nc.scalar.activation(at[:, 0:ACUT], ps[:, 0:ACUT],
                                         AF.Exp, scale=-0.5)
                if QB + n > ACUT:                 # DVE tail, 2-phase
                    w = QB + n - ACUT
                    i1 = strm.tile([128, 192], I32, tag="i1", bufs=2,
                                   name=f"i1_{b}_{qb}_{kb}")
                    i2 = strm.tile([128, 192], I32, tag="i2", bufs=2,
                                   name=f"i2_{b}_{qb}_{kb}")
                    nc.vector.tensor_scalar(
                        i1[:, 0:w], ps[:, ACUT:QB + n], EXP_A, EXP_B1,
                        ALU.mult, ALU.add,
                    )
                    nc.vector.tensor_scalar(
                        i2[:, 0:w], ps[:, ACUT:QB + n], EXP_A, EXP_B2,
                        ALU.mult, ALU.add,
                    )
                    nc.vector.scalar_tensor_tensor(
                        at[:, ACUT:QB + n], i2[:, 0:w].bitcast(FP), EXP_W2,
                        i1[:, 0:w].bitcast(FP), ALU.mult, ALU.add,
                    )
                if j < 4:                         # diagonal: mask last 128
                    nc.vector.tensor_mul(
                        at[:, j * 128:n], at[:, j * 128:n], mask_t[:]
                    )
                    nc.vector.tensor_mul(
                        at[:, QB + j * 128:QB + n],
                        at[:, QB + j * 128:QB + n], mask_t[:]
                    )
                return at

            # ---- ctx pair (both heads, concurrent col tiles) ----
            def emit_ctx(b, qb, kb, pc, at, n, first, last):
                g, go = (b * 16 + kb) // 4, ((b * 16 + kb) % 4) * 128
                m1 = nc.tensor.matmul(
                    pc[0:64, 0:n], Vg[g][:, go:go + 64], at[:, 0:n],
                    start=first, stop=last,
                    tile_position=(0, 0), skip_group_check=True,
                )
                sdep(m1, vg_ci[g])
                m2 = nc.tensor.matmul(
                    pc[64:128, 0:n], Vg[g][:, go + 64:go + 128],
                    at[:, QB:QB + n],
                    start=first, stop=last,
                    tile_position=(0, 64), skip_group_check=True,
                )
                sdep(m2, vg_ci[g])

            # ---------------- prologue: slab 0 projections --------------
            for _, it in proj_items(0):
                it()
            # remaining slabs go through the drip queue (b0's own slabs
            # 1-3 first; they gate early attention iterations, so the
            # early budget is generous)
            for sg in range(1, NSLAB):
                work.extend(proj_items(sg))

            # ---------------- attention main loop, software-pipelined ----
            # Per query row, the diagonal j=3 block (n=512, full width)
            # runs FIRST so the start=True ctx matmul clears the whole
            # 512-col PSUM range: PSUM has_written bits are in an
            # arbitrary state on first execution, and narrower start
            # blocks left cols the later accumulating matmuls touched
            # uninitialized (NaN ctx on the first-ever query row).
            for b in range(B):
                pend = None                       # (qb, kb, first, last)+score
                pc = None
                for qb in range(NQB):
                    seq = ([4 * qb + 3, 4 * qb, 4 * qb + 1, 4 * qb + 2]
                           + list(range(4 * qb + 4, NKB)))
                    for idx, kb in enumerate(seq):
                        fl = (idx == 0, idx == len(seq) - 1)
                        ensure(b * 4 + qb, b * 4 + kb // 4)
                        if pend is None:          # prologue of this batch
                            pend = (qb, kb) + fl + emit_score(b, qb, kb)
                            pc = pcx.tile([128, QB], FP, tag="pc",
                                          name=f"pc{b}_{qb}")
                            drip(6)
                            continue
                        pqb, pkb, pfirst, plast, ps, j, n = pend
                        at = emit_exp(b, pqb, pkb, ps, j, n)
                        # next score pair ahead of this ctx pair
                        pend = (qb, kb) + fl + emit_score(b, qb, kb)
                        if qb != pqb:             # new q row -> new psum
                            pc_next = pcx.tile([128, QB], FP, tag="pc",
                                               name=f"pc{b}_{qb}")
                        emit_ctx(b, pqb, pkb, pc, at, n, pfirst, plast)
                        if qb != pqb:
                            ci = nc.vector.tensor_copy(
                                ctxT[b][:, pqb * QB:(pqb + 1) * QB], pc[:]
                            )
                            work.extend(outproj_items(b, pqb, dep=ci))
                            pc = pc_next
                            # the PE idles on the pc copy at row seams
                            # (2-3us ACT gaps in the trace) -- use them
                            drip(4)
                        # generous budget while projections are pending
                        # (gated by x DMA anyway); 3/iter in steady state
                        drip(9 if b == 0 else 3)
                # drain the last pending iteration
                pqb, pkb, pfirst, plast, ps, j, n = pend
                at = emit_exp(b, pqb, pkb, ps, j, n)
                emit_ctx(b, pqb, pkb, pc, at, n, pfirst, plast)
                ci = nc.vector.tensor_copy(
                    ctxT[b][:, pqb * QB:(pqb + 1) * QB], pc[:]
                )
                work.extend(outproj_items(b, pqb, dep=ci, tail=(b == 1)))

            # tail: whatever is still queued (last out-proj chunk)
            while work:
                work.pop(0)[1]()

            # Completion barrier for the final output DMAs: nothing
            # re-reads outT on-device, so the last ot buffers' DMA
            # semaphores would otherwise never be waited on and the
            # program could "finish" with writes still in flight
            # (observed as garbage tail chunks on first execution).
            # Re-allocating every ot buffer forces a WAR wait on each
            # outstanding DMA's completion semaphore.
            for i in range(3):
                fin = strm.tile([128, 512], BF, tag="ot", bufs=3,
                                name=f"fin{i}")
                nc.vector.tensor_copy(fin[:, 0:8], mask_t[:, 0:8])

            if dbg:
                nc.sync.dma_start(dbg_dT[:], dT[0][:])
                nc.sync.dma_start(dbg_Vg[:], Vg[0][:])
                nc.sync.dma_start(dbg_ctxT[:], ctxT[0][:])
                nc.sync.dma_start(dbg_xa[:], xa[:, 0:4096])
                dchk = res.tile([128, 8], BF, tag="dchk")
                nc.scalar.dma_start(dchk[:], dbg_ctxT[:, 0:8])
                dchk2 = res.tile([128, 8], BF, tag="dchk2")
                nc.vector.tensor_copy(dchk2[:], dchk[:])

    nc.compile()
    return nc


def make_in_maps(inputs):
    x = np.asarray(inputs["x"], np.float32)
    Wq = np.asarray(inputs["Wq"], np.float32)
    Wk = np.asarray(inputs["Wk"], np.float32)
    Wv = np.asarray(inputs["Wv"], np.float32)
    Wo = np.asarray(inputs["Wo"], np.float32)
    # bq/bk/bv are zeros by the problem's input spec; dif = x @ (Wq - Wk)
    # and v = x @ Wv absorb them exactly when zero.  bo is added on host.

    # xs[p, sg*4096 + k*512 + t] = x[token sg*512+t, feature k*128+p]
    xT = x.reshape(T, H).T                       # [feat, tok]
    xs = np.ascontiguousarray(
        xT.reshape(8, 128, NSLAB, 512).transpose(1, 2, 0, 3).reshape(
            128, NSLAB * 4096)
    ).astype(np.float16)
    Wd = Wq - Wk
    maskbf = np.tril(np.ones((128, 128), np.float32), -1).astype(
        ml_dtypes.bfloat16)

    def chunkify(w):                             # [1024, 128] -> [128, 1024]
        return np.ascontiguousarray(
            w.reshape(8, 128, 128).transpose(1, 0, 2).reshape(128, 1024))

    in_maps = []
    for c in range(NC):
        cols = slice(c * 128, (c + 1) * 128)
        in_maps.append({
            "xs": xs,
            "Wdc": chunkify(Wd[:, cols]).astype(np.float16),
            "Wvc": chunkify(Wv[:, cols]).astype(np.float16),
            "Woc": np.ascontiguousarray(Wo[cols, :]).astype(
                ml_dtypes.bfloat16),
            "maskbf": maskbf,
        })
    return in_maps


def gather_out(res, bo):
    acc = np.zeros((H, T), np.float64)
    for c in range(NC):
        acc += np.asarray(res.results[c]["outT"], np.float32)
    return acc.T.reshape(B, S, H).astype(np.float32) + bo


def kernel(**inputs):
    if "nc" not in _cached:
        _cached["nc"] = _build()
    nc = _cached["nc"]
    in_maps = make_in_maps(inputs)
    res = run_bass_kernel_spmd(nc, in_maps, core_ids=list(range(NC)))
    return gather_out(res, np.asarray(inputs["bo"], np.float32))
